# revision 10
# baseline (speedup 1.0000x reference)
"""Local (banded, window=3) attention TRN2 kernel — v3 (engine-rebalanced).

Full-input contract: kernel(**inputs) takes the complete tensors
  x [8, 1024, 384], qkv_w [1152, 384], proj_w [384, 384], proj_b [384]
and returns the full output [8, 1024, 384].

Sharding: data-parallel over batch B=8 -> one batch element per NeuronCore.

v3 changes vs v2 (each targets a measured bottleneck from the v2 trace —
PE busy 45.7us at ~1.4GHz avg, DVE 21.8us with many 1x-mode ops):
  - softmax-denominator path in bf16 (e stripes, indA/i6): the den
    matmuls drop from fp32 (4 cyc/row) to bf16 (1 cyc/row): -6.1k PE cyc
  - p-broadcast via DMA with stride-0 source APs instead of PE matmul +
    ACT evac: -6.1k PE cyc, -12 ACT copies; pb lands in SBUF as bf16 so
    the AV muls run in DVE 2x mode instead of 1x PSUM reads
  - k_odd: a DMA-shifted copy of k at +1 column, so the off-diagonal
    band products q*k_shift are 4B-aligned on both operands (DVE 2x
    instead of 1x): -3us DVE
  - dLs: DMA-shifted copy of dL, so the right-neighbor AV mul is
    aligned too
  - all new DMAs issue from the sync/gpsimd queues (slack) -- never from
    the tensor queue
"""

import os

import numpy as np

KDEBUG = bool(int(os.environ.get("KDEBUG", "0")))

B, N, C = 8, 1024, 384
H, HD = 6, 64
CQKV = 3 * C  # 1152
NCORES = 8
P = 128
NHALF = N // 2  # 512
KC = C // P  # 3 contraction chunks
STAGE_M = (0, 3, 1, 4, 2, 5, 6, 7, 8)  # stage-1 output-chunk order (q/k first)
N_WARM = 36   # narrow matmuls bridging DMA latency (HAM stays warm)

_cached = {}


def _build_nc():
    import contextlib

    import concourse.bacc as bacc
    import concourse.tile as tile
    from concourse import mybir

    f32 = mybir.dt.float32
    bf16 = mybir.dt.bfloat16
    AF = mybir.ActivationFunctionType

    nc = bacc.Bacc("TRN2", target_bir_lowering=False, debug=False,
                   num_devices=NCORES)

    d_x = nc.dram_tensor("xp", [P, KC * N], bf16, kind="ExternalInput").ap()
    d_wq = nc.dram_tensor("wqp", [P, 9 * KC * P], bf16,
                          kind="ExternalInput").ap()
    d_wp = nc.dram_tensor("wpp", [P, KC * KC * P], bf16,
                          kind="ExternalInput").ap()
    d_ind6 = nc.dram_tensor("ind6", [P, 6 * KC], bf16,
                            kind="ExternalInput").ap()
    d_ind6T = nc.dram_tensor("ind6T", [H, P * KC], bf16,
                             kind="ExternalInput").ap()
    d_biasT = nc.dram_tensor("biasT", [P, KC], f32,
                             kind="ExternalInput").ap()
    d_yT = nc.dram_tensor("yT", [C, N], bf16, kind="ExternalOutput").ap()
    if KDEBUG:
        d_dbg_e = [nc.dram_tensor(f"dbg_e{h}", [44, NHALF], bf16,
                                  kind="ExternalOutput").ap()
                   for h in range(2)]
        d_dbg_rec = [nc.dram_tensor(f"dbg_rec{h}", [H, NHALF], f32,
                                    kind="ExternalOutput").ap()
                     for h in range(2)]
        d_dbg_p = [nc.dram_tensor(f"dbg_p{h}", [2 * H, NHALF], bf16,
                                  kind="ExternalOutput").ap()
                   for h in range(2)]
        d_dbg_pb = nc.dram_tensor("dbg_pb", [P, N], bf16,
                                  kind="ExternalOutput").ap()
        d_dbg_qkv = nc.dram_tensor("dbg_qkv", [CQKV, N], bf16,
                                   kind="ExternalOutput").ap()

    with tile.TileContext(nc) as tc, contextlib.ExitStack() as ctx:
        wpool = ctx.enter_context(tc.tile_pool(name="w", bufs=1))
        xpool = ctx.enter_context(tc.tile_pool(name="x", bufs=1))
        qkvpool = ctx.enter_context(tc.tile_pool(name="qkv", bufs=1))
        prodpool = ctx.enter_context(tc.tile_pool(name="prod", bufs=9))
        avpool = ctx.enter_context(tc.tile_pool(name="av", bufs=4))
        ypool = ctx.enter_context(tc.tile_pool(name="y", bufs=1))
        epool = ctx.enter_context(tc.tile_pool(name="e", bufs=2))
        # PSUM budget (8 banks of 512 fp32):
        #   mm [128,512] x3 (stage-1 qkv, then the proj accumulators)
        #   pb [128,512] x2 (warmup, then p-broadcast)
        #   s  [6,512]   x2 (scores)
        #   fill [128,512] x1 (warmup; never read)
        mmpool = ctx.enter_context(
            tc.tile_pool(name="mm", bufs=3, space="PSUM"))
        pbpool = ctx.enter_context(
            tc.tile_pool(name="pb", bufs=2, space="PSUM"))
        fillpool = ctx.enter_context(
            tc.tile_pool(name="fill", bufs=1, space="PSUM"))
        spool = ctx.enter_context(
            tc.tile_pool(name="s", bufs=2, space="PSUM"))

        # ---- input DMAs: big contiguous transfers on the two HWDGE queues
        xt = xpool.tile([P, KC * N], bf16, name="xt")
        # first halves of all three c-chunks first so the first stage-1
        # group (h=0) can start as soon as possible
        nc.sync.dma_start(
            out=xt.rearrange("p (k n) -> p k n", k=KC)[:, :, 0:NHALF],
            in_=d_x.rearrange("p (k n) -> p k n", k=KC)[:, :, 0:NHALF])

        wqt = wpool.tile([P, 9 * KC * P], bf16, name="wqt")
        # qk-chunk weights (stages 0-5) first on scalar, then x piece B,
        # then the v-chunk weights
        nc.scalar.dma_start(out=wqt[:, 0:2 * KC * P],
                            in_=d_wq[:, 0:2 * KC * P])
        nc.scalar.dma_start(out=wqt[:, 2 * KC * P:6 * KC * P],
                            in_=d_wq[:, 2 * KC * P:6 * KC * P])
        nc.scalar.dma_start(
            out=xt.rearrange("p (k n) -> p k n", k=KC)[:, :, NHALF:N],
            in_=d_x.rearrange("p (k n) -> p k n", k=KC)[:, :, NHALF:N])
        nc.scalar.dma_start(out=wqt[:, 6 * KC * P:], in_=d_wq[:, 6 * KC * P:])

        ind6 = wpool.tile([P, 6 * KC], bf16, name="ind6")
        nc.sync.dma_start(out=ind6, in_=d_ind6)
        ind6T = wpool.tile([H, P * KC], bf16, name="ind6T")
        nc.sync.dma_start(out=ind6T, in_=d_ind6T)
        biasT = wpool.tile([P, KC], f32, name="biasT")
        nc.sync.dma_start(out=biasT, in_=d_biasT)
        wpt = wpool.tile([P, KC * KC * P], bf16, name="wpt")
        nc.scalar.dma_start(out=wpt, in_=d_wp)

        # ---- PE warm-up ------------------------------------------------
        # TRN2's activity monitor runs the PE at 1.2 GHz until it has been
        # continuously busy ~3.4us, and drops back after idle gaps. Narrow
        # dummy matmuls (128-col rhs, 1-col weights) keep it busy from boot
        # until the first input tiles land, so the real stream runs at 2.4.
        warm = wpool.tile([P, NHALF], bf16, name="warm")
        nc.gpsimd.memset(warm, 0.0)

        fill_t = [None]

        def pe_fill(n):
            # narrow matmuls into a dedicated PSUM bank (never read, never
            # shared) — keeps the PE activity monitor warm across latency
            # bubbles without coupling to live rings
            if fill_t[0] is None:
                fill_t[0] = fillpool.tile([P, NHALF], f32, tag="f",
                                          name="fillps")
            for _ in range(n):
                nc.tensor.matmul(fill_t[0][0:32, 0:P], lhsT=warm[:, 0:32],
                                 rhs=warm[:, 0:P], start=True, stop=True)

        pe_fill(N_WARM)

        def x_sl(kc, h):
            return xt[:, N * kc + NHALF * h:N * kc + NHALF * (h + 1)]

        # ---- stage 1: qkvT chunks --------------------------------------
        qkvT = [None] * 9

        def stage1_chunk(s, h):
            m = STAGE_M[s]
            if qkvT[m] is None:
                qkvT[m] = qkvpool.tile([P, N], bf16, name=f"qkvT{m}")
            ps = mmpool.tile([P, NHALF], f32, tag="mm")
            for kc in range(KC):
                nc.tensor.matmul(
                    ps,
                    lhsT=wqt[:, KC * P * s + P * kc:KC * P * s + P * (kc + 1)],
                    rhs=x_sl(kc, h),
                    start=(kc == 0), stop=(kc == KC - 1),
                )
            dst = qkvT[m][:, NHALF * h:NHALF * (h + 1)]
            if s >= 6:
                nc.vector.tensor_copy(dst, ps)
            else:
                nc.scalar.copy(dst, ps)

        # piece A (first halves of all kc) feeds the h=0 groups; h=1
        # groups start once piece B lands
        for s, h in ((0, 0), (1, 0), (2, 0), (3, 0), (0, 1), (1, 1),
                     (4, 0), (5, 0), (2, 1), (3, 1), (4, 1), (5, 1)):
            stage1_chunk(s, h)

        # ---- k_odd: +1-column-shifted copies of the k chunks (DMA) -----
        # Gives the off-diagonal products a 4B-aligned second operand so
        # the DVE runs them in 2x mode. Zero padding cols 0 and N+1 make
        # the boundary products come out zero (masked after exp anyway).
        k_odd = [None] * KC
        for kc in range(KC):
            ko = prodpool.tile([P, N + 2], bf16, tag="kodd",
                               bufs=3, name=f"kodd{kc}")
            nc.gpsimd.memset(ko[:, 0:1], 0.0)
            nc.gpsimd.memset(ko[:, N + 1:N + 2], 0.0)
            nc.sync.dma_start(out=ko[:, 1:N + 1], in_=qkvT[3 + kc][:, 0:N])
            k_odd[kc] = ko

        # ---- DVE: band products (emitted early; run as soon as q/k land)
        def make_prod(off, kc):
            q = qkvT[kc]
            pr = prodpool.tile([P, N], bf16, tag="prod",
                               name=f"prod{off}_{kc}")
            if off == 0:
                # k_odd[:, t] = k[t-1]; col 0 is zero padding
                nc.vector.tensor_mul(pr, q, k_odd[kc][:, 0:N])
            elif off == 1:
                nc.vector.tensor_mul(pr, q, qkvT[3 + kc])
            else:
                # k_odd[:, t+2] = k[t+1]; col N-1 reads the zero padding
                nc.vector.tensor_mul(pr, q, k_odd[kc][:, 2:N + 2])
            return pr

        prods = [[None] * KC for _ in range(3)]
        # off=1 first (no k_odd dependency): runs while the k_odd DMAs fly
        for off, kc in ((1, 0), (1, 1), (1, 2), (0, 0), (2, 0),
                        (0, 1), (2, 1), (0, 2), (2, 2)):
            prods[off][kc] = make_prod(off, kc)

        # dL[t] = v[t-1] - v[t] (dL[0] = 0); dLs[t] = dL[t+1] (dLs[N-1]=0)
        # attn = v + p_l*dL - p_r*dLs. dL chains on gpsimd (SBUF-only);
        # dLs is a DMA-shifted copy so the AV mul stays aligned.
        dLs_ = [None] * KC
        dL_ = [None] * KC

        def make_dL(kc):
            v = qkvT[6 + kc]
            dL = avpool.tile([P, N], bf16, tag="dv", bufs=3,
                             name=f"dL{kc}")
            nc.gpsimd.memset(dL[:, 0:1], 0.0)
            nc.gpsimd.tensor_sub(dL[:, 1:N], v[:, 0:N - 1], v[:, 1:N])
            dLs = avpool.tile([P, N], bf16, tag="dvs", bufs=3,
                              name=f"dLs{kc}")
            nc.gpsimd.memset(dLs[:, N - 1:N], 0.0)
            nc.sync.dma_start(out=dLs[:, 0:N - 1], in_=dL[:, 1:N])
            dL_[kc] = dL
            dLs_[kc] = dLs

        # e tiles (bf16, base partition 0): one [6, NHALF] tile per
        # (offset, half). The softmax denominator is then two DVE adds per
        # half -- no PE matmuls, no partition stripes.
        e_t = [[None] * 2 for _ in range(3)]
        for off in range(3):
            for h in range(2):
                e_t[off][h] = epool.tile([H, NHALF], bf16, tag="e", bufs=6,
                                         name=f"e{off}_{h}")

        scale = float(HD) ** -0.5

        def scores(h):
            for off in range(3):
                sps = spool.tile([H, NHALF], f32, tag="s")
                for kc in range(KC):
                    nc.tensor.matmul(
                        sps,
                        lhsT=ind6[:, 6 * kc:6 * (kc + 1)],
                        rhs=prods[off][kc][:, NHALF * h:NHALF * (h + 1)],
                        start=(kc == 0), stop=(kc == KC - 1),
                    )
                with tc.high_priority():
                    nc.scalar.activation(e_t[off][h], sps, AF.Exp,
                                         scale=scale)
            # boundary mask, inline right after the exps:
            # no left neighbor at t=0 (h=0), no right neighbor at N-1 (h=1)
            with tc.high_priority():
                if h == 0:
                    nc.vector.memset(e_t[0][0][0:H, 0:1], 0.0)
                else:
                    nc.vector.memset(e_t[2][1][0:H, NHALF - 1:NHALF], 0.0)

        def v_chunk(s):
            stage1_chunk(s, 0)
            stage1_chunk(s, 1)

        # v chunks interleaved with score matmuls: keeps the PE busy while
        # the DVE finishes prods / the ACT runs exps
        v_chunk(6)
        scores(0)
        make_dL(0)
        v_chunk(7)
        scores(1)
        make_dL(1)
        v_chunk(8)
        make_dL(2)

        # softmax denominator: two DVE adds per half (bf16 2x mode)
        den_t = [None, None]

        def den(h):
            t = epool.tile([H, NHALF], bf16, tag="dent", bufs=2)
            nc.vector.tensor_add(t, e_t[0][h], e_t[1][h])
            d = epool.tile([H, NHALF], f32, tag="den", bufs=2,
                           name=f"den{h}")
            nc.vector.tensor_add(d, t, e_t[2][h])
            den_t[h] = d

        den(0)

        # reciprocal + p = e * rec  (p in bf16 for the broadcast matmul)
        p_half = [[None, None] for _ in range(2)]  # [h][0 -> off0, 1 -> off2]
        recs_dbg = [None, None]

        def softmax(h):
            with tc.high_priority():
                rec = epool.tile([H, NHALF], f32, tag="rec", bufs=2)
                recs_dbg[h] = rec
                nc.vector.reciprocal_approx_fast(out=rec, in_=den_t[h])
                for i, src_e in enumerate((e_t[0][h], e_t[2][h])):
                    pt = epool.tile([H, NHALF], bf16, tag="p", bufs=4,
                                    name=f"p{h}_{i}")
                    nc.vector.tensor_mul(pt, src_e, rec)
                    p_half[h][i] = pt

        softmax(0)

        # ---- p broadcast (PE) + AV (DVE 2x on SBUF bf16) ---------------
        ybuf = [ypool.tile([P, KC * NHALF], bf16, name=f"ybuf{h}")
                for h in range(2)]

        def bcast(h, i, kc):
            # broadcast into PSUM, evacuate to bf16 SBUF on ACT right away
            pbps = pbpool.tile([P, NHALF], f32, tag="pb")
            nc.tensor.matmul(
                pbps,
                lhsT=ind6T[:, P * kc:P * (kc + 1)],
                rhs=p_half[h][i],
                start=True, stop=True,
            )
            pbs = avpool.tile([P, NHALF], bf16, tag="pbs", bufs=6,
                              name=f"pbs{h}_{i}_{kc}")
            nc.scalar.copy(pbs, pbps)
            return pbs

        def av_chain(h, kc, pb0, pb2):
            lo = NHALF * h
            hi = lo + NHALF
            m1 = avpool.tile([P, NHALF], bf16, tag="m", bufs=4)
            nc.vector.tensor_mul(m1, pb0, dL_[kc][:, lo:hi])
            m2 = avpool.tile([P, NHALF], bf16, tag="m", bufs=4)
            nc.vector.tensor_mul(m2, pb2, dLs_[kc][:, lo:hi])
            u = avpool.tile([P, NHALF], bf16, tag="u", bufs=4,
                            name=f"u{kc}_{h}")
            nc.vector.tensor_sub(u, m1, m2)
            return u

        def proj_v(h, yps):
            # yT = Wp @ v + Wp @ u: the v half runs while the softmax/AV
            # chain computes u, keeping the PE busy (and HAM warm)
            lo = NHALF * h
            hi = lo + NHALF
            for kc in range(KC):
                for m in range(KC):
                    nc.tensor.matmul(
                        yps[m],
                        lhsT=wpt[:,
                                 KC * P * m + P * kc:KC * P * m + P * (kc + 1)],
                        rhs=qkvT[6 + kc][:, lo:hi],
                        start=(kc == 0), stop=False,
                    )

        def proj_u(kc, u, yps):
            for m in range(KC):
                nc.tensor.matmul(
                    yps[m],
                    lhsT=wpt[:, KC * P * m + P * kc:KC * P * m + P * (kc + 1)],
                    rhs=u,
                    start=False, stop=(kc == KC - 1),
                )

        pb_dbg = [None]
        for h in range(2):
            yps = [mmpool.tile([P, NHALF], f32, tag="mm", name=f"y{m}_{h}")
                   for m in range(KC)]
            proj_v(h, yps)
            if h == 0:
                den(1)
            pb = {}
            pb[0] = (bcast(h, 0, 0), bcast(h, 1, 0))
            pb[1] = (bcast(h, 0, 1), bcast(h, 1, 1))
            pb[2] = (bcast(h, 0, 2), bcast(h, 1, 2))
            if pb_dbg[0] is None:
                pb_dbg[0] = pb[0][0]
            u0 = av_chain(h, 0, *pb[0])
            if h == 0:
                softmax(1)
            pe_fill(5)
            proj_u(0, u0, yps)
            u1 = av_chain(h, 1, *pb[1])
            pe_fill(4)
            proj_u(1, u1, yps)
            u2 = av_chain(h, 2, *pb[2])
            pe_fill(4)
            proj_u(2, u2, yps)

            for m in range(KC):
                nc.scalar.add(ybuf[h][:, NHALF * m:NHALF * (m + 1)], yps[m],
                              biasT[:, m:m + 1])
                nc.sync.dma_start(
                    out=d_yT[P * m:P * (m + 1),
                             NHALF * h:NHALF * (h + 1)],
                    in_=ybuf[h][:, NHALF * m:NHALF * (m + 1)])

        if KDEBUG:
            for h in range(2):
                for off in range(3):
                    nc.sync.dma_start(out=d_dbg_e[h][6 * off:6 * off + 6, :],
                                      in_=e_t[off][h])
                nc.sync.dma_start(out=d_dbg_rec[h], in_=recs_dbg[h])
                nc.sync.dma_start(out=d_dbg_p[h][0:H, :],
                                  in_=p_half[h][0])
                nc.sync.dma_start(out=d_dbg_p[h][H:2 * H, :],
                                  in_=p_half[h][1])
            nc.sync.dma_start(out=d_dbg_pb[:, 0:NHALF], in_=pb_dbg[0])
            for m in range(9):
                nc.sync.dma_start(out=d_dbg_qkv[P * m:P * (m + 1), :],
                                  in_=qkvT[m])

    nc.compile()
    return nc


def _host_inputs(x, qkv_w, proj_w, proj_b):
    import ml_dtypes
    bf = ml_dtypes.bfloat16

    qkv_w = qkv_w.astype(np.float32)
    proj_w = proj_w.astype(np.float32)

    # wq packed per stage chunk: [p, s*384 + kc*128 + i]
    #   = qkv_w[128*STAGE_M[s] + i, 128*kc + p]
    wq = np.empty((P, 9 * KC * P), np.float32)
    for s, m in enumerate(STAGE_M):
        blk = qkv_w[P * m:P * (m + 1), :]  # [i=128, c=384]
        t = blk.T.reshape(KC, P, P).transpose(1, 0, 2).reshape(P, KC * P)
        wq[:, KC * P * s:KC * P * (s + 1)] = t
    wp = np.empty((P, KC * KC * P), np.float32)
    for m in range(KC):
        blk = proj_w[P * m:P * (m + 1), :]
        t = blk.T.reshape(KC, P, P).transpose(1, 0, 2).reshape(P, KC * P)
        wp[:, KC * P * m:KC * P * (m + 1)] = t

    ind6 = np.zeros((P, 6 * KC), np.float32)
    ind6T = np.zeros((H, P * KC), np.float32)
    for kc in range(KC):
        for p in range(P):
            ind6[p, 6 * kc + 2 * kc + p // HD] = 1.0
            ind6T[2 * kc + p // HD, P * kc + p] = 1.0
    biasT = proj_b.astype(np.float32).reshape(KC, P).T.copy()

    shared = {
        "wqp": wq.astype(bf),
        "wpp": wp.astype(bf),
        "ind6": ind6.astype(bf),
        "ind6T": ind6T.astype(bf),
        "biasT": np.ascontiguousarray(biasT),
    }
    in_maps = []
    for b in range(B):
        m = dict(shared)
        xT = x[b].astype(np.float32).T  # [C, N]
        m["xp"] = np.ascontiguousarray(
            xT.reshape(KC, P, N).transpose(1, 0, 2).reshape(P, KC * N)
        ).astype(bf)
        in_maps.append(m)
    return in_maps


def kernel(x, qkv_w, proj_w, proj_b, _trace=False):
    from concourse import bass_utils

    x = np.asarray(x)
    if "nc" not in _cached:
        _cached["nc"] = _build_nc()
    nc = _cached["nc"]
    in_maps = _host_inputs(x, np.asarray(qkv_w), np.asarray(proj_w),
                           np.asarray(proj_b))
    res = bass_utils.run_bass_kernel_spmd(
        nc, in_maps, core_ids=list(range(NCORES)), trace=_trace)
    out = np.empty((B, N, C), np.float32)
    for b in range(B):
        out[b] = res.results[b]["yT"].astype(np.float32).T
    if _trace:
        _cached["last_result"] = res
    return out


# revision 15
# speedup vs baseline: 1.1698x; 1.1698x over previous
"""Local (banded, window=3) attention TRN2 kernel — v3 (engine-rebalanced).

Full-input contract: kernel(**inputs) takes the complete tensors
  x [8, 1024, 384], qkv_w [1152, 384], proj_w [384, 384], proj_b [384]
and returns the full output [8, 1024, 384].

Sharding: data-parallel over batch B=8 -> one batch element per NeuronCore.

v3 changes vs v2 (each targets a measured bottleneck from the v2 trace —
PE busy 45.7us at ~1.4GHz avg, DVE 21.8us with many 1x-mode ops):
  - softmax-denominator path in bf16 (e stripes, indA/i6): the den
    matmuls drop from fp32 (4 cyc/row) to bf16 (1 cyc/row): -6.1k PE cyc
  - p-broadcast via DMA with stride-0 source APs instead of PE matmul +
    ACT evac: -6.1k PE cyc, -12 ACT copies; pb lands in SBUF as bf16 so
    the AV muls run in DVE 2x mode instead of 1x PSUM reads
  - k_odd: a DMA-shifted copy of k at +1 column, so the off-diagonal
    band products q*k_shift are 4B-aligned on both operands (DVE 2x
    instead of 1x): -3us DVE
  - dLs: DMA-shifted copy of dL, so the right-neighbor AV mul is
    aligned too
  - all new DMAs issue from the sync/gpsimd queues (slack) -- never from
    the tensor queue
"""

import os

import numpy as np

KDEBUG = bool(int(os.environ.get("KDEBUG", "0")))

B, N, C = 8, 1024, 384
H, HD = 6, 64
CQKV = 3 * C  # 1152
NCORES = 8
P = 128
NHALF = N // 2  # 512
KC = C // P  # 3 contraction chunks
STAGE_M = (0, 3, 1, 4, 2, 5, 6, 7, 8)  # stage-1 output-chunk order (q/k first)
N_WARM = 36   # narrow matmuls bridging DMA latency (HAM stays warm)

_cached = {}


def _build_nc():
    import contextlib

    import concourse.bacc as bacc
    import concourse.tile as tile
    from concourse import mybir

    f32 = mybir.dt.float32
    bf16 = mybir.dt.bfloat16
    AF = mybir.ActivationFunctionType

    nc = bacc.Bacc("TRN2", target_bir_lowering=False, debug=False,
                   num_devices=NCORES)

    d_x = nc.dram_tensor("xp", [P, KC * N], bf16, kind="ExternalInput").ap()
    d_wq = nc.dram_tensor("wqp", [P, 9 * KC * P], bf16,
                          kind="ExternalInput").ap()
    d_wp = nc.dram_tensor("wpp", [P, KC * KC * P], bf16,
                          kind="ExternalInput").ap()
    d_ind6 = nc.dram_tensor("ind6", [P, 6 * KC], bf16,
                            kind="ExternalInput").ap()
    d_ind6T = nc.dram_tensor("ind6T", [H, P * KC], bf16,
                             kind="ExternalInput").ap()
    d_i6 = nc.dram_tensor("i6", [H, H], bf16, kind="ExternalInput").ap()
    d_biasT = nc.dram_tensor("biasT", [P, KC], f32,
                             kind="ExternalInput").ap()
    d_yT = nc.dram_tensor("yT", [C, N], bf16, kind="ExternalOutput").ap()
    if KDEBUG:
        d_dbg_e = [nc.dram_tensor(f"dbg_e{h}", [44, NHALF], bf16,
                                  kind="ExternalOutput").ap()
                   for h in range(2)]
        d_dbg_rec = [nc.dram_tensor(f"dbg_rec{h}", [H, NHALF], f32,
                                    kind="ExternalOutput").ap()
                     for h in range(2)]
        d_dbg_p = [nc.dram_tensor(f"dbg_p{h}", [2 * H, NHALF], bf16,
                                  kind="ExternalOutput").ap()
                   for h in range(2)]
        d_dbg_pb = nc.dram_tensor("dbg_pb", [P, N], bf16,
                                  kind="ExternalOutput").ap()
        d_dbg_qkv = nc.dram_tensor("dbg_qkv", [CQKV, N], bf16,
                                   kind="ExternalOutput").ap()

    with tile.TileContext(nc) as tc, contextlib.ExitStack() as ctx:
        wpool = ctx.enter_context(tc.tile_pool(name="w", bufs=1))
        xpool = ctx.enter_context(tc.tile_pool(name="x", bufs=1))
        qkvpool = ctx.enter_context(tc.tile_pool(name="qkv", bufs=1))
        prodpool = ctx.enter_context(tc.tile_pool(name="prod", bufs=9))
        avpool = ctx.enter_context(tc.tile_pool(name="av", bufs=4))
        ypool = ctx.enter_context(tc.tile_pool(name="y", bufs=1))
        epool = ctx.enter_context(tc.tile_pool(name="e", bufs=2))
        # PSUM budget (8 banks of 512 fp32):
        #   mm [128,512] x3 (stage-1 qkv, then the proj accumulators)
        #   pb [128,512] x2 (warmup, then p-broadcast)
        #   s  [6,512]   x2 (scores)
        #   fill [128,512] x1 (warmup; never read)
        mmpool = ctx.enter_context(
            tc.tile_pool(name="mm", bufs=3, space="PSUM"))
        pbpool = ctx.enter_context(
            tc.tile_pool(name="pb", bufs=2, space="PSUM"))
        fillpool = ctx.enter_context(
            tc.tile_pool(name="fill", bufs=1, space="PSUM"))
        spool = ctx.enter_context(
            tc.tile_pool(name="s", bufs=2, space="PSUM"))

        # ---- input DMAs: big contiguous transfers on the two HWDGE queues
        xt = xpool.tile([P, KC * N], bf16, name="xt")
        # first halves of all three c-chunks first so the first stage-1
        # group (h=0) can start as soon as possible
        nc.sync.dma_start(
            out=xt.rearrange("p (k n) -> p k n", k=KC)[:, :, 0:NHALF],
            in_=d_x.rearrange("p (k n) -> p k n", k=KC)[:, :, 0:NHALF])

        wqt = wpool.tile([P, 9 * KC * P], bf16, name="wqt")
        # qk-chunk weights (stages 0-5) first on scalar, then x piece B,
        # then the v-chunk weights
        nc.scalar.dma_start(out=wqt[:, 0:2 * KC * P],
                            in_=d_wq[:, 0:2 * KC * P])
        nc.scalar.dma_start(out=wqt[:, 2 * KC * P:6 * KC * P],
                            in_=d_wq[:, 2 * KC * P:6 * KC * P])
        nc.scalar.dma_start(
            out=xt.rearrange("p (k n) -> p k n", k=KC)[:, :, NHALF:N],
            in_=d_x.rearrange("p (k n) -> p k n", k=KC)[:, :, NHALF:N])
        nc.scalar.dma_start(out=wqt[:, 6 * KC * P:], in_=d_wq[:, 6 * KC * P:])

        ind6 = wpool.tile([P, 6 * KC], bf16, name="ind6")
        nc.sync.dma_start(out=ind6, in_=d_ind6)
        ind6T = wpool.tile([H, P * KC], bf16, name="ind6T")
        nc.sync.dma_start(out=ind6T, in_=d_ind6T)
        i6 = wpool.tile([H, H], bf16, name="i6")
        nc.sync.dma_start(out=i6, in_=d_i6)
        biasT = wpool.tile([P, KC], f32, name="biasT")
        nc.sync.dma_start(out=biasT, in_=d_biasT)
        wpt = wpool.tile([P, KC * KC * P], bf16, name="wpt")
        nc.scalar.dma_start(out=wpt, in_=d_wp)

        # ---- PE warm-up ------------------------------------------------
        # TRN2's activity monitor runs the PE at 1.2 GHz until it has been
        # continuously busy ~3.4us, and drops back after idle gaps. Narrow
        # dummy matmuls (128-col rhs, 1-col weights) keep it busy from boot
        # until the first input tiles land, so the real stream runs at 2.4.
        warm = wpool.tile([P, NHALF], bf16, name="warm")
        nc.gpsimd.memset(warm, 0.0)

        fill_t = [None]

        def pe_fill(n):
            # narrow matmuls into a dedicated PSUM bank (never read, never
            # shared) — keeps the PE activity monitor warm across latency
            # bubbles without coupling to live rings
            if fill_t[0] is None:
                fill_t[0] = fillpool.tile([P, NHALF], f32, tag="f",
                                          name="fillps")
            for _ in range(n):
                nc.tensor.matmul(fill_t[0][0:32, 0:P], lhsT=warm[:, 0:32],
                                 rhs=warm[:, 0:P], start=True, stop=True)

        pe_fill(N_WARM)

        def x_sl(kc, h):
            return xt[:, N * kc + NHALF * h:N * kc + NHALF * (h + 1)]

        # ---- stage 1: qkvT chunks --------------------------------------
        qkvT = [None] * 9

        def stage1_chunk(s, h):
            m = STAGE_M[s]
            if qkvT[m] is None:
                qkvT[m] = qkvpool.tile([P, N], bf16, name=f"qkvT{m}")
            ps = mmpool.tile([P, NHALF], f32, tag="mm")
            for kc in range(KC):
                nc.tensor.matmul(
                    ps,
                    lhsT=wqt[:, KC * P * s + P * kc:KC * P * s + P * (kc + 1)],
                    rhs=x_sl(kc, h),
                    start=(kc == 0), stop=(kc == KC - 1),
                )
            dst = qkvT[m][:, NHALF * h:NHALF * (h + 1)]
            if s >= 6:
                nc.vector.tensor_copy(dst, ps)
            else:
                nc.scalar.copy(dst, ps)

        # piece A (first halves of all kc) feeds the h=0 groups; h=1
        # groups start once piece B lands
        for s, h in ((0, 0), (1, 0), (2, 0), (3, 0), (0, 1), (1, 1),
                     (4, 0), (5, 0), (2, 1), (3, 1), (4, 1), (5, 1)):
            stage1_chunk(s, h)

        # ---- k_odd: +1-column-shifted copies of the k chunks (DMA) -----
        # Gives the off-diagonal products a 4B-aligned second operand so
        # the DVE runs them in 2x mode. Zero padding cols 0 and N+1 make
        # the boundary products come out zero (masked after exp anyway).
        k_odd = [None] * KC
        for kc in range(KC):
            ko = prodpool.tile([P, N + 2], bf16, tag="kodd",
                               bufs=3, name=f"kodd{kc}")
            nc.gpsimd.memset(ko[:, 0:1], 0.0)
            nc.gpsimd.memset(ko[:, N + 1:N + 2], 0.0)
            nc.sync.dma_start(out=ko[:, 1:N + 1], in_=qkvT[3 + kc][:, 0:N])
            k_odd[kc] = ko

        # ---- DVE: band products (emitted early; run as soon as q/k land)
        def make_prod(off, kc):
            q = qkvT[kc]
            pr = prodpool.tile([P, N], bf16, tag="prod",
                               name=f"prod{off}_{kc}")
            if off == 0:
                # k_odd[:, t] = k[t-1]; col 0 is zero padding
                nc.vector.tensor_mul(pr, q, k_odd[kc][:, 0:N])
            elif off == 1:
                nc.vector.tensor_mul(pr, q, qkvT[3 + kc])
            else:
                # k_odd[:, t+2] = k[t+1]; col N-1 reads the zero padding
                nc.vector.tensor_mul(pr, q, k_odd[kc][:, 2:N + 2])
            return pr

        prods = [[None] * KC for _ in range(3)]
        # off=1 first (no k_odd dependency): runs while the k_odd DMAs fly
        for off, kc in ((1, 0), (1, 1), (1, 2), (0, 0), (2, 0),
                        (0, 1), (2, 1), (0, 2), (2, 2)):
            prods[off][kc] = make_prod(off, kc)

        # dL[t] = v[t-1] - v[t] (dL[0] = 0); dLs[t] = dL[t+1] (dLs[N-1]=0)
        # attn = v + p_l*dL - p_r*dLs. dL chains on gpsimd (SBUF-only);
        # dLs is a DMA-shifted copy so the AV mul stays aligned.
        dLs_ = [None] * KC
        dL_ = [None] * KC

        def make_dL(kc):
            v = qkvT[6 + kc]
            dL = avpool.tile([P, N], bf16, tag="dv", bufs=3,
                             name=f"dL{kc}")
            nc.gpsimd.memset(dL[:, 0:1], 0.0)
            nc.gpsimd.tensor_sub(dL[:, 1:N], v[:, 0:N - 1], v[:, 1:N])
            dLs = avpool.tile([P, N], bf16, tag="dvs", bufs=3,
                              name=f"dLs{kc}")
            nc.gpsimd.memset(dLs[:, N - 1:N], 0.0)
            nc.sync.dma_start(out=dLs[:, 0:N - 1], in_=dL[:, 1:N])
            dL_[kc] = dL
            dLs_[kc] = dLs

        # e tiles (bf16, base partition 0): one [6, NHALF] tile per
        # (offset, half). The softmax denominator is then two DVE adds per
        # half -- no PE matmuls, no partition stripes.
        e_t = [[None] * 2 for _ in range(3)]
        for off in range(3):
            for h in range(2):
                e_t[off][h] = epool.tile([H, NHALF], bf16, tag="e", bufs=6,
                                         name=f"e{off}_{h}")

        scale = float(HD) ** -0.5

        def scores(h):
            for off in range(3):
                sps = spool.tile([H, NHALF], f32, tag="s")
                for kc in range(KC):
                    nc.tensor.matmul(
                        sps,
                        lhsT=ind6[:, 6 * kc:6 * (kc + 1)],
                        rhs=prods[off][kc][:, NHALF * h:NHALF * (h + 1)],
                        start=(kc == 0), stop=(kc == KC - 1),
                    )
                with tc.high_priority():
                    nc.scalar.activation(e_t[off][h], sps, AF.Exp,
                                         scale=scale)
            # boundary mask, inline right after the exps:
            # no left neighbor at t=0 (h=0), no right neighbor at N-1 (h=1)
            with tc.high_priority():
                if h == 0:
                    nc.vector.memset(e_t[0][0][0:H, 0:1], 0.0)
                else:
                    nc.vector.memset(e_t[2][1][0:H, NHALF - 1:NHALF], 0.0)

        def v_chunk(s):
            stage1_chunk(s, 0)
            stage1_chunk(s, 1)

        # v chunks interleaved with score matmuls: keeps the PE busy while
        # the DVE finishes prods / the ACT runs exps
        v_chunk(6)
        scores(0)
        make_dL(0)
        v_chunk(7)
        scores(1)
        make_dL(1)
        v_chunk(8)
        make_dL(2)

        # softmax denominator: two accumulated bf16 matmuls per half on the
        # PE (keeps the chain off the in-order DVE queue; bf16 indicator
        # matmuls are 1 cyc/row vs fp32's 4)
        den_t = [None, None]

        def den(h):
            dps = spool.tile([H, NHALF], f32, tag="s", name=f"den{h}")
            for off in range(3):
                nc.tensor.matmul(dps, lhsT=i6, rhs=e_t[off][h],
                                 start=(off == 0), stop=(off == 2))
            den_t[h] = dps

        den(0)

        # reciprocal + p = e * rec  (p in bf16 for the broadcast matmul)
        p_half = [[None, None] for _ in range(2)]  # [h][0 -> off0, 1 -> off2]
        recs_dbg = [None, None]

        def softmax(h):
            with tc.high_priority():
                rec = epool.tile([H, NHALF], f32, tag="rec", bufs=2)
                recs_dbg[h] = rec
                nc.vector.reciprocal_approx_fast(out=rec, in_=den_t[h])
                for i, src_e in enumerate((e_t[0][h], e_t[2][h])):
                    pt = epool.tile([H, NHALF], bf16, tag="p", bufs=4,
                                    name=f"p{h}_{i}")
                    nc.vector.tensor_mul(pt, src_e, rec)
                    p_half[h][i] = pt

        softmax(0)

        # ---- p broadcast (PE) + AV (DVE 2x on SBUF bf16) ---------------
        ybuf = [ypool.tile([P, KC * NHALF], bf16, name=f"ybuf{h}")
                for h in range(2)]

        def bcast(h, i, kc):
            # broadcast into PSUM, evacuate to bf16 SBUF on ACT right away
            pbps = pbpool.tile([P, NHALF], f32, tag="pb")
            nc.tensor.matmul(
                pbps,
                lhsT=ind6T[:, P * kc:P * (kc + 1)],
                rhs=p_half[h][i],
                start=True, stop=True,
            )
            pbs = avpool.tile([P, NHALF], bf16, tag="pbs", bufs=6,
                              name=f"pbs{h}_{i}_{kc}")
            nc.scalar.copy(pbs, pbps)
            return pbs

        def av_chain(h, kc, pb0, pb2):
            lo = NHALF * h
            hi = lo + NHALF
            m1 = avpool.tile([P, NHALF], bf16, tag="m", bufs=4)
            nc.vector.tensor_mul(m1, pb0, dL_[kc][:, lo:hi])
            m2 = avpool.tile([P, NHALF], bf16, tag="m", bufs=4)
            nc.vector.tensor_mul(m2, pb2, dLs_[kc][:, lo:hi])
            u = avpool.tile([P, NHALF], bf16, tag="u", bufs=4,
                            name=f"u{kc}_{h}")
            nc.vector.tensor_sub(u, m1, m2)
            return u

        def proj_v(h, yps):
            # yT = Wp @ v + Wp @ u: the v half runs while the softmax/AV
            # chain computes u, keeping the PE busy (and HAM warm)
            lo = NHALF * h
            hi = lo + NHALF
            for kc in range(KC):
                for m in range(KC):
                    nc.tensor.matmul(
                        yps[m],
                        lhsT=wpt[:,
                                 KC * P * m + P * kc:KC * P * m + P * (kc + 1)],
                        rhs=qkvT[6 + kc][:, lo:hi],
                        start=(kc == 0), stop=False,
                    )

        def proj_u(kc, u, yps):
            for m in range(KC):
                nc.tensor.matmul(
                    yps[m],
                    lhsT=wpt[:, KC * P * m + P * kc:KC * P * m + P * (kc + 1)],
                    rhs=u,
                    start=False, stop=(kc == KC - 1),
                )

        pb_dbg = [None]
        for h in range(2):
            yps = [mmpool.tile([P, NHALF], f32, tag="mm", name=f"y{m}_{h}")
                   for m in range(KC)]
            proj_v(h, yps)
            if h == 0:
                den(1)
            pb = {}
            pb[0] = (bcast(h, 0, 0), bcast(h, 1, 0))
            pb[1] = (bcast(h, 0, 1), bcast(h, 1, 1))
            pb[2] = (bcast(h, 0, 2), bcast(h, 1, 2))
            if pb_dbg[0] is None:
                pb_dbg[0] = pb[0][0]
            u0 = av_chain(h, 0, *pb[0])
            if h == 0:
                softmax(1)
            pe_fill(5)
            proj_u(0, u0, yps)
            u1 = av_chain(h, 1, *pb[1])
            pe_fill(4)
            proj_u(1, u1, yps)
            u2 = av_chain(h, 2, *pb[2])
            pe_fill(4)
            proj_u(2, u2, yps)

            for m in range(KC):
                nc.scalar.add(ybuf[h][:, NHALF * m:NHALF * (m + 1)], yps[m],
                              biasT[:, m:m + 1])
                nc.sync.dma_start(
                    out=d_yT[P * m:P * (m + 1),
                             NHALF * h:NHALF * (h + 1)],
                    in_=ybuf[h][:, NHALF * m:NHALF * (m + 1)])

        if KDEBUG:
            for h in range(2):
                for off in range(3):
                    nc.sync.dma_start(out=d_dbg_e[h][6 * off:6 * off + 6, :],
                                      in_=e_t[off][h])
                nc.sync.dma_start(out=d_dbg_rec[h], in_=recs_dbg[h])
                nc.sync.dma_start(out=d_dbg_p[h][0:H, :],
                                  in_=p_half[h][0])
                nc.sync.dma_start(out=d_dbg_p[h][H:2 * H, :],
                                  in_=p_half[h][1])
            nc.sync.dma_start(out=d_dbg_pb[:, 0:NHALF], in_=pb_dbg[0])
            for m in range(9):
                nc.sync.dma_start(out=d_dbg_qkv[P * m:P * (m + 1), :],
                                  in_=qkvT[m])

    nc.compile()
    return nc


def _host_inputs(x, qkv_w, proj_w, proj_b):
    import ml_dtypes
    bf = ml_dtypes.bfloat16

    qkv_w = qkv_w.astype(np.float32)
    proj_w = proj_w.astype(np.float32)

    # wq packed per stage chunk: [p, s*384 + kc*128 + i]
    #   = qkv_w[128*STAGE_M[s] + i, 128*kc + p]
    wq = np.empty((P, 9 * KC * P), np.float32)
    for s, m in enumerate(STAGE_M):
        blk = qkv_w[P * m:P * (m + 1), :]  # [i=128, c=384]
        t = blk.T.reshape(KC, P, P).transpose(1, 0, 2).reshape(P, KC * P)
        wq[:, KC * P * s:KC * P * (s + 1)] = t
    wp = np.empty((P, KC * KC * P), np.float32)
    for m in range(KC):
        blk = proj_w[P * m:P * (m + 1), :]
        t = blk.T.reshape(KC, P, P).transpose(1, 0, 2).reshape(P, KC * P)
        wp[:, KC * P * m:KC * P * (m + 1)] = t

    ind6 = np.zeros((P, 6 * KC), np.float32)
    ind6T = np.zeros((H, P * KC), np.float32)
    for kc in range(KC):
        for p in range(P):
            ind6[p, 6 * kc + 2 * kc + p // HD] = 1.0
            ind6T[2 * kc + p // HD, P * kc + p] = 1.0
    biasT = proj_b.astype(np.float32).reshape(KC, P).T.copy()

    shared = {
        "wqp": wq.astype(bf),
        "wpp": wp.astype(bf),
        "ind6": ind6.astype(bf),
        "ind6T": ind6T.astype(bf),
        "i6": np.eye(H, dtype=np.float32).astype(bf),
        "biasT": np.ascontiguousarray(biasT),
    }
    in_maps = []
    for b in range(B):
        m = dict(shared)
        xT = x[b].astype(np.float32).T  # [C, N]
        m["xp"] = np.ascontiguousarray(
            xT.reshape(KC, P, N).transpose(1, 0, 2).reshape(P, KC * N)
        ).astype(bf)
        in_maps.append(m)
    return in_maps


def kernel(x, qkv_w, proj_w, proj_b, _trace=False):
    from concourse import bass_utils

    x = np.asarray(x)
    if "nc" not in _cached:
        _cached["nc"] = _build_nc()
    nc = _cached["nc"]
    in_maps = _host_inputs(x, np.asarray(qkv_w), np.asarray(proj_w),
                           np.asarray(proj_b))
    res = bass_utils.run_bass_kernel_spmd(
        nc, in_maps, core_ids=list(range(NCORES)), trace=_trace)
    out = np.empty((B, N, C), np.float32)
    for b in range(B):
        out[b] = res.results[b]["yT"].astype(np.float32).T
    if _trace:
        _cached["last_result"] = res
    return out
